# Initial kernel scaffold
#
"""Trainium2 Bass kernel for nn_CoupleClusterLoss.

Reference semantics (N=8192 samples, D=128, 1024 ids):
    mask[i,j] = (t_i == t_j); centers = row-normalized mask @ inputs
    dist2[i,j] = ||center_i - x_j||^2
    dist_cp[i] = max_{j: t_j==t_i} dist2[i,j]; dist_cn[i] = min_{j: t_j!=t_i} dist2[i,j]
    loss = mean(relu(dist_cp - dist_cn + 0.3)); prec = mean(dist_cn > dist_cp)

Key reduction: center_i depends only on the class c = t_i, so dist_cp/dist_cn are
per-class quantities: cp[c] = max_{j in c} dist2(center_c, x_j), cn[c] = min_{j not in c}.
Also dist2 = c2[c] + x2[j] - 2*center_c.x_j and the c2[c] term is constant per class so
it cancels in cp[c] - cn[c].  The loss only needs diff[c] = cp[c] - cn[c] and the class
counts n_c:
    loss = sum_c n_c * relu(diff_c + margin) / N;  prec = sum_c n_c * (diff_c < 0) / N

Sharding: 8 cores x 128 classes each.  Per core (class block c0 = 128*core):
  - onehotT[j, c] = (targets[j] == c0+c) built from an iota row + per-partition compare
  - class sums + counts: PSUM accumulation of matmul(lhsT=onehotT tile, rhs=[inputs|1])
  - A[c, j] = x2[j] - 2*center_c.x_j + BIG*onehot[c,j] accumulated in PSUM by 3 matmuls
    (centers row, ones x2 row, BIG*Identity with onehotT stationary)
  - cp[c] = max_j A - BIG, cn[c] = min_j A  =>  device returns maxA - minA and counts;
    host subtracts BIG and does the tiny weighted reduction.
"""

import numpy as np

import concourse.bass as bass
import concourse.tile as tile
from concourse import masks, mybir
from concourse.bass_utils import run_bass_kernel_spmd

N = 8192          # samples
D = 128           # feature dim
NIDS = 1024       # number of identities
NCORES = 8
CPC = NIDS // NCORES   # classes per core = 128
NT = N // 128          # j tiles = 64
CHUNK = 512            # dist-matrix free-dim chunk
NCHUNK = N // CHUNK    # 16
TPC = CHUNK // 128     # onehot tiles per chunk = 4
BIG = 16384.0
MARGIN = 0.3
F32 = mybir.dt.float32


def _split_wide_ctrl_waits(nc, maxw=1):
    """walrus on this container rejects CTRL-type instructions (Drain) carrying
    more than a couple of semaphore waits ("Too many sync wait commands").
    Split any wide Drain into a chain of Drains with <= maxw waits each."""
    for fn in nc.m.functions:
        for blk in fn.blocks:
            newlist = []
            for ins in blk.instructions:
                si = ins.sync_info
                if si is not None and len(si.on_wait) > maxw and ins.opcode == "Drain":
                    waits = list(si.on_wait)
                    chunks = [waits[i:i + maxw] for i in range(0, len(waits), maxw)]
                    for ci, chunk in enumerate(chunks[:-1]):
                        carrier = mybir.InstDrain(
                            name=f"{ins.name}-presplit{ci}", ins=[], outs=[])
                        carrier.engine = ins.engine
                        carrier.sync_info = mybir.SyncInfo(on_wait=chunk, on_update=[])
                        carrier.debug = ins.debug
                        newlist.append(carrier)
                    ins.sync_info = mybir.SyncInfo(
                        on_wait=chunks[-1], on_update=list(si.on_update))
                newlist.append(ins)
            blk.instructions = newlist


def _build_bass():
    nc = bass.Bass()
    x_in = nc.declare_dram_parameter("inputs", [N, D], F32, isOutput=False)
    # targets_t[p, t] = targets[t*128 + p] - 128*core   (host-prepared, f32)
    tt_in = nc.declare_dram_parameter("targets_t", [128, NT], F32, isOutput=False)
    diff_out = nc.declare_dram_parameter("diff", [128, 1], F32, isOutput=True)
    cnts_out = nc.declare_dram_parameter("counts", [128, 1], F32, isOutput=True)

    with tile.TileContext(nc) as tc:
        with tc.sbuf_pool(name="persist", bufs=1) as pp, \
             tc.sbuf_pool(name="scratch", bufs=2) as sp, \
             tc.psum_pool(name="pst", bufs=2) as pst, \
             tc.psum_pool(name="psd", bufs=2) as psd:

            # ---- persistent SBUF buffers -------------------------------------
            xa = pp.tile([128, NT, D + 1], F32, name="xa")     # [p, t, d|1] aug inputs
            xT = pp.tile([128, N], F32, name="xT")             # inputs^T [d, j]
            oh = pp.tile([128, N], F32, name="oh")             # onehotT [j_p, t*128+c]... [j, c] per tile
            tt = pp.tile([128, NT], F32, name="tt")            # shifted targets
            iota_f = pp.tile([128, 128], F32, name="iota_f")   # row 0..127 on every partition
            ident = pp.tile([128, 128], F32, name="ident")
            bigid = pp.tile([128, 128], F32, name="bigid")
            ones1p = pp.tile([1, 128], F32, name="ones1p")
            x2cols = pp.tile([128, NT], F32, name="x2cols")    # x2 in [p, t] layout
            x2colsT = pp.tile([64, 128], F32, name="x2colsT")
            x2row = pp.tile([1, N], F32, name="x2row")         # x2 as a single row
            maxcols = pp.tile([128, NCHUNK], F32, name="maxcols")
            mincols = pp.tile([128, NCHUNK], F32, name="mincols")
            cnts = pp.tile([128, 1], F32, name="cnts")
            recip = pp.tile([128, 1], F32, name="recip")
            n2recip = pp.tile([128, 1], F32, name="n2recip")
            n2c = pp.tile([128, 128], F32, name="n2c")         # -2*centers [c, d]
            n2cT = pp.tile([128, 128], F32, name="n2cT")       # [d, c]
            cpv = pp.tile([128, 1], F32, name="cpv")
            cnv = pp.tile([128, 1], F32, name="cnv")
            diffv = pp.tile([128, 1], F32, name="diffv")

            # ---- constants ---------------------------------------------------
            iota_i = sp.tile([128, 128], mybir.dt.int32, name="iota_i")
            nc.gpsimd.iota(iota_i[:], pattern=[[1, 128]], base=0, channel_multiplier=0)
            nc.vector.tensor_copy(iota_f[:], iota_i[:])
            masks.make_identity(nc, ident[:])
            nc.vector.tensor_scalar_mul(bigid[:], ident[:], BIG)
            nc.vector.memset(ones1p[:], 1.0)

            # ---- input loads -------------------------------------------------
            nc.sync.dma_start(out=tt[:], in_=tt_in[:])
            xa3 = xa  # [128, NT, D+1]
            for g in range(16):  # 4 j-tiles per DMA
                nc.sync.dma_start(
                    out=xa3[:, g * 4:(g + 1) * 4, 0:D],
                    in_=x_in[g * 512:(g + 1) * 512, :].rearrange(
                        "(t p) d -> p t d", p=128),
                )
            nc.gpsimd.memset(xa3[:, :, D], 1.0)

            # ---- per-tile: onehotT, transpose, x2 ----------------------------
            ps1 = pst.tile([128, D + 1], F32, name="ps1", bufs=1)
            for t in range(NT):
                oh_t = oh[:, t * 128:(t + 1) * 128]
                nc.vector.tensor_scalar(
                    out=oh_t, in0=iota_f[:], scalar1=tt[:, t:t + 1], scalar2=None,
                    op0=mybir.AluOpType.is_equal)
                # class sums + counts accumulate: out[c, d|cnt]
                nc.tensor.matmul(
                    out=ps1[:], lhsT=oh_t, rhs=xa3[:, t, :],
                    start=(t == 0), stop=(t == NT - 1))
                # inputs^T tile via PE transpose
                ps_tr = pst.tile([128, 128], F32, name="ps_tr")
                nc.tensor.transpose(ps_tr[:], xa3[:, t, 0:D], ident[:])
                eng = nc.scalar if (t % 2 == 0) else nc.vector
                eng.tensor_copy(xT[:, t * 128:(t + 1) * 128], ps_tr[:])
                # x2 for this tile (sum over d of x^2), [128, 1] column
                sq_scr = sp.tile([128, D], F32, name="sq_scr")
                nc.vector.tensor_tensor_reduce(
                    out=sq_scr[:], in0=xa3[:, t, 0:D], in1=xa3[:, t, 0:D],
                    scale=1.0, scalar=0.0,
                    op0=mybir.AluOpType.mult, op1=mybir.AluOpType.add,
                    accum_out=x2cols[:, t:t + 1])

            # ---- centers -----------------------------------------------------
            nc.vector.tensor_copy(cnts[:], ps1[:, D:D + 1])
            nc.vector.reciprocal(recip[:], cnts[:])
            nc.vector.tensor_scalar_mul(n2recip[:], recip[:], -2.0)
            nc.vector.tensor_scalar(
                out=n2c[:], in0=ps1[:, 0:D], scalar1=n2recip[:], scalar2=None,
                op0=mybir.AluOpType.mult)
            ps_c = pst.tile([128, 128], F32, name="ps_c")
            nc.tensor.transpose(ps_c[:], n2c[:], ident[:])
            nc.vector.tensor_copy(n2cT[:], ps_c[:])

            # ---- x2 row: [128, NT] -> [64, 128] -> [1, N] --------------------
            ps_x2 = pst.tile([64, 128], F32, name="ps_x2")
            nc.tensor.matmul(out=ps_x2[:], lhsT=x2cols[:], rhs=ident[:],
                             is_transpose=True)
            nc.vector.tensor_copy(x2colsT[:], ps_x2[:])
            nc.sync.dma_start(
                out=x2row[0:1, :].rearrange("a (t p) -> a t p", p=128),
                in_=x2colsT[:])

            # ---- distance row-block, chunked over j --------------------------
            for k in range(NCHUNK):
                psD = psd.tile([128, CHUNK], F32, name="psD")
                nc.tensor.matmul(
                    out=psD[:], lhsT=n2cT[:], rhs=xT[:, k * CHUNK:(k + 1) * CHUNK],
                    start=True, stop=False)
                nc.tensor.matmul(
                    out=psD[:], lhsT=ones1p[:],
                    rhs=x2row[0:1, k * CHUNK:(k + 1) * CHUNK],
                    start=False, stop=False, skip_group_check=True)
                for s in range(TPC):
                    t = TPC * k + s
                    nc.tensor.matmul(
                        out=psD[:, s * 128:(s + 1) * 128],
                        lhsT=oh[:, t * 128:(t + 1) * 128], rhs=bigid[:],
                        start=False, stop=(s == TPC - 1), skip_group_check=True)
                nc.vector.tensor_reduce(
                    out=maxcols[:, k:k + 1], in_=psD[:],
                    axis=mybir.AxisListType.X, op=mybir.AluOpType.max)
                nc.vector.tensor_reduce(
                    out=mincols[:, k:k + 1], in_=psD[:],
                    axis=mybir.AxisListType.X, op=mybir.AluOpType.min)

            # ---- epilogue ----------------------------------------------------
            nc.vector.tensor_reduce(out=cpv[:], in_=maxcols[:],
                                    axis=mybir.AxisListType.X, op=mybir.AluOpType.max)
            nc.vector.tensor_reduce(out=cnv[:], in_=mincols[:],
                                    axis=mybir.AxisListType.X, op=mybir.AluOpType.min)
            nc.vector.tensor_tensor(out=diffv[:], in0=cpv[:], in1=cnv[:],
                                    op=mybir.AluOpType.subtract)
            nc.sync.dma_start(out=diff_out[:], in_=diffv[:])
            nc.sync.dma_start(out=cnts_out[:], in_=cnts[:])

    _split_wide_ctrl_waits(nc)
    return nc


_NC = None


def _get_nc():
    global _NC
    if _NC is None:
        _NC = _build_bass()
    return _NC


def kernel(**inputs):
    x = np.ascontiguousarray(np.asarray(inputs["inputs"], dtype=np.float32))
    t = np.asarray(inputs["targets"]).astype(np.int64)
    assert x.shape == (N, D) and t.shape == (N,)

    # targets arranged [p, t] so tile[p, jt] = targets[jt*128 + p]; shifted per core
    tt_base = t.reshape(NT, 128).T.astype(np.float32)  # [128, NT]
    in_maps = []
    for core in range(NCORES):
        in_maps.append({
            "inputs": x,
            "targets_t": np.ascontiguousarray(tt_base - np.float32(core * CPC)),
        })

    nc = _get_nc()
    res = run_bass_kernel_spmd(nc, in_maps, list(range(NCORES)))

    diffs = np.concatenate(
        [res.results[i]["diff"][:, 0] for i in range(NCORES)]).astype(np.float64)
    cnts = np.concatenate(
        [res.results[i]["counts"][:, 0] for i in range(NCORES)]).astype(np.float64)

    diffs = diffs - BIG  # cp - cn per class
    valid = cnts > 0
    dv = diffs[valid]
    cv = cnts[valid]
    loss = np.sum(cv * np.maximum(dv + MARGIN, 0.0)) / N
    prec = np.sum(cv * (dv < 0.0)) / N
    return np.float32(loss), np.float32(prec)


# revision 7
# speedup vs baseline: 1.4080x; 1.4080x over previous
"""Trainium2 Bass kernel for nn_CoupleClusterLoss.

Reference semantics (N=8192 samples, D=128, 1024 ids):
    mask[i,j] = (t_i == t_j); centers = row-normalized mask @ inputs
    dist2[i,j] = ||center_i - x_j||^2
    dist_cp[i] = max_{j: t_j==t_i} dist2[i,j]; dist_cn[i] = min_{j: t_j!=t_i} dist2[i,j]
    loss = mean(relu(dist_cp - dist_cn + 0.3)); prec = mean(dist_cn > dist_cp)

Key reduction: center_i depends only on the class c = t_i, so dist_cp/dist_cn are
per-class quantities: cp[c] = max_{j in c} dist2(center_c, x_j), cn[c] = min_{j not in c}.
Also dist2 = c2[c] + x2[j] - 2*center_c.x_j and the c2[c] term is constant per class so
it cancels in cp[c] - cn[c].  The loss only needs diff[c] = cp[c] - cn[c] and the class
counts n_c:
    loss = sum_c n_c * relu(diff_c + margin) / N;  prec = sum_c n_c * (diff_c < 0) / N

Sharding: 8 cores x 128 classes each.  Per core (class block c0 = 128*core):
  - onehotT[j, c] = (targets[j] == c0+c) built from an iota row + per-partition compare
  - class sums + counts: PSUM accumulation of matmul(lhsT=onehotT tile, rhs=[inputs|1])
  - A[c, j] = x2[j] - 2*center_c.x_j + BIG*onehot[c,j] accumulated in PSUM by 3 matmuls
    (centers row, ones x2 row, BIG*Identity with onehotT stationary)
  - cp[c] = max_j A - BIG, cn[c] = min_j A  =>  device returns maxA - minA and counts;
    host subtracts BIG and does the tiny weighted reduction.
"""

import numpy as np

import concourse.bass as bass
import concourse.tile as tile
from concourse import masks, mybir
from concourse.bass_utils import run_bass_kernel_spmd

N = 8192          # samples
D = 128           # feature dim
NIDS = 1024       # number of identities
NCORES = 8
CPC = NIDS // NCORES   # classes per core = 128
NT = N // 128          # j tiles = 64
CHUNK = 512            # dist-matrix free-dim chunk
NCHUNK = N // CHUNK    # 16
TPC = CHUNK // 128     # onehot tiles per chunk = 4
BIG = 16384.0
MARGIN = 0.3
F32 = mybir.dt.float32


def _split_wide_ctrl_waits(nc, maxw=1):
    """walrus on this container rejects instructions carrying more than one
    semaphore wait ("Too many sync wait commands", both CTRL and compute
    encodings).  Keep one wait on the instruction and move the rest onto
    same-engine NoOp carriers inserted right before it."""
    for fn in nc.m.functions:
        for blk in fn.blocks:
            newlist = []
            for ins in blk.instructions:
                si = ins.sync_info
                if si is not None and len(si.on_wait) > maxw:
                    waits = list(si.on_wait)
                    chunks = [waits[i:i + maxw] for i in range(0, len(waits), maxw)]
                    for ci, chunk in enumerate(chunks[:-1]):
                        carrier = mybir.InstNoOp(
                            name=f"{ins.name}-presplit{ci}", ins=[], outs=[])
                        carrier.engine = ins.engine
                        carrier.sync_info = mybir.SyncInfo(on_wait=chunk, on_update=[])
                        carrier.debug = ins.debug
                        newlist.append(carrier)
                    ins.sync_info = mybir.SyncInfo(
                        on_wait=chunks[-1], on_update=list(si.on_update))
                newlist.append(ins)
            blk.instructions = newlist


def _build_bass():
    nc = bass.Bass()
    x_in = nc.declare_dram_parameter("inputs", [N, D], F32, isOutput=False)
    # targets_t[p, t] = targets[t*128 + p] - 128*core   (host-prepared, f32)
    tt_in = nc.declare_dram_parameter("targets_t", [128, NT], F32, isOutput=False)
    diff_out = nc.declare_dram_parameter("diff", [128, 1], F32, isOutput=True)
    cnts_out = nc.declare_dram_parameter("counts", [128, 1], F32, isOutput=True)

    with tile.TileContext(nc) as tc:
        with tc.sbuf_pool(name="persist", bufs=1) as pp, \
             tc.sbuf_pool(name="scratch", bufs=2) as sp, \
             tc.psum_pool(name="pst", bufs=2) as pst, \
             tc.psum_pool(name="psd", bufs=3) as psd:

            # ---- persistent SBUF buffers -------------------------------------
            xa = pp.tile([128, NT, D + 1], F32, name="xa")     # [p, t, d|1] aug inputs
            xT = pp.tile([128, N], F32, name="xT")             # inputs^T [d, j]
            oh = pp.tile([128, N], F32, name="oh")             # onehotT [j_p, t*128+c]... [j, c] per tile
            tt = pp.tile([128, NT], F32, name="tt")            # shifted targets
            iota_f = pp.tile([128, 128], F32, name="iota_f")   # row 0..127 on every partition
            ident = pp.tile([128, 128], F32, name="ident")
            bigid = pp.tile([128, 128], F32, name="bigid")
            ones1p = pp.tile([1, 128], F32, name="ones1p")
            x2cols = pp.tile([128, NT], F32, name="x2cols")    # x2 in [p, t] layout
            x2colsT = pp.tile([64, 128], F32, name="x2colsT")
            x2row = pp.tile([1, N], F32, name="x2row")         # x2 as a single row
            maxcols = pp.tile([128, NCHUNK], F32, name="maxcols")
            mincols = pp.tile([128, NCHUNK], F32, name="mincols")
            cnts = pp.tile([128, 1], F32, name="cnts")
            recip = pp.tile([128, 1], F32, name="recip")
            n2recip = pp.tile([128, 1], F32, name="n2recip")
            n2c = pp.tile([128, 128], F32, name="n2c")         # -2*centers [c, d]
            n2cT = pp.tile([128, 128], F32, name="n2cT")       # [d, c]
            cpv = pp.tile([128, 1], F32, name="cpv")
            cnv = pp.tile([128, 1], F32, name="cnv")
            diffv = pp.tile([128, 1], F32, name="diffv")

            # ---- constants ---------------------------------------------------
            iota_i = sp.tile([128, 128], mybir.dt.int32, name="iota_i")
            nc.gpsimd.iota(iota_i[:], pattern=[[1, 128]], base=0, channel_multiplier=0)
            nc.vector.tensor_copy(iota_f[:], iota_i[:])
            masks.make_identity(nc, ident[:])
            nc.vector.tensor_scalar_mul(bigid[:], ident[:], BIG)
            nc.vector.memset(ones1p[:], 1.0)

            # ---- input loads -------------------------------------------------
            nc.sync.dma_start(out=tt[:], in_=tt_in[:])
            xa3 = xa  # [128, NT, D+1]
            for g in range(16):  # 4 j-tiles per DMA
                nc.sync.dma_start(
                    out=xa3[:, g * 4:(g + 1) * 4, 0:D],
                    in_=x_in[g * 512:(g + 1) * 512, :].rearrange(
                        "(t p) d -> p t d", p=128),
                )
            nc.gpsimd.memset(xa3[:, :, D], 1.0)

            # ---- per-tile: onehotT, transpose, x2 ----------------------------
            ps1 = pst.tile([128, D + 1], F32, name="ps1", bufs=1)
            for t in range(NT):
                oh_t = oh[:, t * 128:(t + 1) * 128]
                nc.vector.tensor_scalar(
                    out=oh_t, in0=iota_f[:], scalar1=tt[:, t:t + 1], scalar2=None,
                    op0=mybir.AluOpType.is_equal)
                # class sums + counts accumulate: out[c, d|cnt]
                nc.tensor.matmul(
                    out=ps1[:], lhsT=oh_t, rhs=xa3[:, t, :],
                    start=(t == 0), stop=(t == NT - 1))
                # inputs^T tile via PE transpose
                ps_tr = pst.tile([128, 128], F32, name="ps_tr", tag="tr")
                nc.tensor.transpose(ps_tr[:], xa3[:, t, 0:D], ident[:])
                if t % 2 == 0:
                    nc.scalar.copy(xT[:, t * 128:(t + 1) * 128], ps_tr[:])
                else:
                    nc.vector.tensor_copy(xT[:, t * 128:(t + 1) * 128], ps_tr[:])
                # x2 for this tile (sum over d of x^2), [128, 1] column
                sq_scr = sp.tile([128, D], F32, name="sq_scr")
                nc.scalar.activation(
                    out=sq_scr[:], in_=xa3[:, t, 0:D],
                    func=mybir.ActivationFunctionType.Square,
                    accum_out=x2cols[:, t:t + 1])

            # ---- centers -----------------------------------------------------
            nc.vector.tensor_copy(cnts[:], ps1[:, D:D + 1])
            nc.vector.reciprocal(recip[:], cnts[:])
            nc.vector.tensor_scalar_mul(n2recip[:], recip[:], -2.0)
            nc.vector.tensor_scalar(
                out=n2c[:], in0=ps1[:, 0:D], scalar1=n2recip[:], scalar2=None,
                op0=mybir.AluOpType.mult)
            ps_c = pst.tile([128, 128], F32, name="ps_c", tag="tr")
            nc.tensor.transpose(ps_c[:], n2c[:], ident[:])
            nc.vector.tensor_copy(n2cT[:], ps_c[:])

            # ---- x2 row: [128, NT] -> [64, 128] -> [1, N] --------------------
            ps_x2 = pst.tile([64, 128], F32, name="ps_x2", tag="tr")
            nc.tensor.matmul(out=ps_x2[:], lhsT=x2cols[:], rhs=ident[:],
                             is_transpose=True)
            nc.vector.tensor_copy(x2colsT[:], ps_x2[:])
            nc.sync.dma_start(
                out=x2row[0:1, :].rearrange("a (t p) -> a t p", p=128),
                in_=x2colsT[:])

            # ---- distance row-block, chunked over j --------------------------
            for k in range(NCHUNK):
                psD = psd.tile([128, CHUNK], F32, name="psD")
                nc.tensor.matmul(
                    out=psD[:], lhsT=n2cT[:], rhs=xT[:, k * CHUNK:(k + 1) * CHUNK],
                    start=True, stop=False)
                nc.tensor.matmul(
                    out=psD[:], lhsT=ones1p[:],
                    rhs=x2row[0:1, k * CHUNK:(k + 1) * CHUNK],
                    start=False, stop=False, skip_group_check=True)
                for s in range(TPC):
                    t = TPC * k + s
                    nc.tensor.matmul(
                        out=psD[:, s * 128:(s + 1) * 128],
                        lhsT=oh[:, t * 128:(t + 1) * 128], rhs=bigid[:],
                        start=False, stop=(s == TPC - 1), skip_group_check=True)
                nc.vector.tensor_reduce(
                    out=maxcols[:, k:k + 1], in_=psD[:],
                    axis=mybir.AxisListType.X, op=mybir.AluOpType.max)
                nc.vector.tensor_reduce(
                    out=mincols[:, k:k + 1], in_=psD[:],
                    axis=mybir.AxisListType.X, op=mybir.AluOpType.min)

            # ---- epilogue ----------------------------------------------------
            nc.vector.tensor_reduce(out=cpv[:], in_=maxcols[:],
                                    axis=mybir.AxisListType.X, op=mybir.AluOpType.max)
            nc.vector.tensor_reduce(out=cnv[:], in_=mincols[:],
                                    axis=mybir.AxisListType.X, op=mybir.AluOpType.min)
            nc.vector.tensor_tensor(out=diffv[:], in0=cpv[:], in1=cnv[:],
                                    op=mybir.AluOpType.subtract)
            nc.sync.dma_start(out=diff_out[:], in_=diffv[:])
            nc.sync.dma_start(out=cnts_out[:], in_=cnts[:])

    _split_wide_ctrl_waits(nc)
    return nc


_NC = None


def _get_nc():
    global _NC
    if _NC is None:
        _NC = _build_bass()
    return _NC


def kernel(**inputs):
    x = np.ascontiguousarray(np.asarray(inputs["inputs"], dtype=np.float32))
    t = np.asarray(inputs["targets"]).astype(np.int64)
    assert x.shape == (N, D) and t.shape == (N,)

    # targets arranged [p, t] so tile[p, jt] = targets[jt*128 + p]; shifted per core
    tt_base = t.reshape(NT, 128).T.astype(np.float32)  # [128, NT]
    in_maps = []
    for core in range(NCORES):
        in_maps.append({
            "inputs": x,
            "targets_t": np.ascontiguousarray(tt_base - np.float32(core * CPC)),
        })

    nc = _get_nc()
    res = run_bass_kernel_spmd(nc, in_maps, list(range(NCORES)))

    diffs = np.concatenate(
        [res.results[i]["diff"][:, 0] for i in range(NCORES)]).astype(np.float64)
    cnts = np.concatenate(
        [res.results[i]["counts"][:, 0] for i in range(NCORES)]).astype(np.float64)

    diffs = diffs - BIG  # cp - cn per class
    valid = cnts > 0
    dv = diffs[valid]
    cv = cnts[valid]
    loss = np.sum(cv * np.maximum(dv + MARGIN, 0.0)) / N
    prec = np.sum(cv * (dv < 0.0)) / N
    return np.float32(loss), np.float32(prec)


def _install_ntff_hook():
    """The agent image's antenv lacks axon_hooks; synthesize it from the boot
    helper so run_bass_kernel_spmd(trace=True) can capture NTFF profiles."""
    import sys
    import types
    if "antenv.axon_hooks" in sys.modules:
        return
    import trn_agent_boot.trn_boot as tb
    hook = tb._ntff_profile_via_ctypes("/opt/axon/libaxon_pjrt.so")
    mod = types.ModuleType("antenv.axon_hooks")
    mod.get_axon_ntff_profile_hook = lambda: hook
    sys.modules["antenv.axon_hooks"] = mod


def timed_run(np_inputs, tmpdir=None):
    """Dev helper (not used by the grader): run once with NTFF tracing and
    return HW exec time in ns."""
    _install_ntff_hook()
    import concourse.bass_utils as bu
    bu.upload_artifacts = lambda d: f"local:{d}"  # no bucket creds in container
    x = np.ascontiguousarray(np.asarray(np_inputs["inputs"], dtype=np.float32))
    t = np.asarray(np_inputs["targets"]).astype(np.int64)
    tt_base = t.reshape(NT, 128).T.astype(np.float32)
    in_maps = [{
        "inputs": x,
        "targets_t": np.ascontiguousarray(tt_base - np.float32(core * CPC)),
    } for core in range(NCORES)]
    nc = _get_nc()
    res = run_bass_kernel_spmd(nc, in_maps, list(range(NCORES)),
                               trace=True, tmpdir=tmpdir)
    return res.exec_time_ns
